# revision 5
# baseline (speedup 1.0000x reference)
"""Trainium2 Bass kernel for nn_LookupLanguageModel (trigram backoff LM lookup).

v3: real HW's indirect DMA consumes ONE offset per partition (multi-offset APs
are a CoreSim-only fiction), so instead of merging gathers by offset we merge
them by HOST TABLE LAYOUT: the read-only trie is repacked (host-side, query-
independent) into per-node records so every dependency level is a single
[128,1]-offset gather:

  TU1[u] (104 i32): num_children, 32 child ids, 32 pre-scaled TB offsets
      ((first_child+i-U)*20), 32 child backoff logs   -- keyed by h1
  TU2[u] (80 i32): 8 slot blocks of [4 child ids, 4 child logs, num_children,
      bw1]                                            -- keyed by h2, slot s
  TB[j-U] (20 i32): 8 trigram child ids, their 8 logs, num_children
      -- keyed by the matched bigram node j

Chain: hconst DMA -> G_A=TU1[h1] + G_B=TU2[h2] (parallel) -> match h2 among
h1's children (masked mult+reduce selects: ids/offsets>=0 via max, logs<0 via
min) -> G_C=TB[j] -> five 128-offset correction scatters (trigram slot +
4 bigram candidates per partition; invalid/collided slots pushed out of range
and dropped via bounds_check). Baseline rows logs[v]+bconst are built from a
host-replicated logs[0:V] (sharding hint: replicate the read-only arrays) and
written out by Sync as soon as bconst is known, overlapping G_C and the
correction math, so the scatters only wait on their own operands.

Layout: 128 partitions = 16 rows x 8 slots; partition p: row b=p>>3 (batch row
16*core+b), slot s=p&7 (output chunk [1024s,1024s+1024), trigram candidate s,
bigram candidates 4s..4s+3).
"""

import numpy as np

import concourse.bass as bass
import concourse.mybir as mybir
from concourse.bass import IndirectOffsetOnAxis
from concourse.bass_utils import run_bass_kernel_spmd

# ---- problem constants (must match the reference trie shapes) ----
V = 8192
N = 3
U = V + 1                   # 8193 unigram nodes
C2, C3 = 32, 8
B2 = U * C2                 # 262176 bigram nodes
B3 = B2 * C3                # 2097408 trigram nodes
XP = U + B2 + 1             # pointers length 270370
KI = B2 + B3                # ids length 2359584
NNODES = U + B2 + B3        # 2367777 (start of backoff weights inside logs)
LL = 2 * XP + (B3 - 1)      # logs length 2638147
BATCH = 128
NCORES = 8
BPC = BATCH // NCORES       # 16 rows per core

W1 = 104                    # TU1 record width
W2 = 80                     # TU2 record width
W3 = 20                     # TB record width

BIG = 1 << 18               # offset mask-out constant (> BPC*V - 1)
BOUNDS = BPC * V - 1        # max valid flat output element index per core

i32 = mybir.dt.int32
f32 = mybir.dt.float32

AX = mybir.AxisListType
OP = mybir.AluOpType

# hconst column map (int32 [128, HC])
HC_IDXA = 0      # h1 * W1
HC_IDXB = 1      # h2 * W2 + 10*s
HC_H2 = 2
HC_IOTA32 = 3    # 32 cols: 0..31
HC_MS8 = 35      # 8 cols: (k==s)
HC_CI4 = 43      # 4 cols: 4s+k
HC_OFFBC = 47    # (b<<13) + BIG
HC_OFFB2C = 48   # (b<<13) + 2*BIG
HC_SCOL = 49     # s
HC = 50

# TU1 record layout
A_NUM = 0
A_C1 = 1         # 32 child ids
A_TBC = 33       # 32 pre-scaled TB offsets
A_BW2 = 65       # 32 child backoff logs (f32 bits)

# TU2 slot-block layout (10 per slot)
B_BI = 0         # 4 bigram candidate ids
B_BL = 4         # 4 bigram candidate logs (f32 bits)
B_NUM2 = 8
B_BW1 = 9

# TB record layout
C_TI = 0         # 8 trigram candidate ids
C_TL = 8         # 8 trigram candidate logs (f32 bits)
C_NUM3 = 16


def build_kernel() -> bass.Bass:
    nc = bass.Bass()

    hconst = nc.declare_dram_parameter("hconst", [128, HC], i32, isOutput=False)
    lurep = nc.declare_dram_parameter("lurep", [128, 1024], f32, isOutput=False)
    tu1 = nc.declare_dram_parameter("tu1", [U * W1, 1], i32, isOutput=False)
    tu2 = nc.declare_dram_parameter("tu2", [U * W2, 1], i32, isOutput=False)
    tb = nc.declare_dram_parameter("tb", [B2 * W3, 1], i32, isOutput=False)
    outp = nc.declare_dram_parameter("out", [BPC * V, 1], f32, isOutput=True)

    from contextlib import ExitStack

    with ExitStack() as ctx:
        _n = [0]

        def sb(shape, dt):
            _n[0] += 1
            return ctx.enter_context(nc.sbuf_tensor(f"t{_n[0]}", shape, dt))

        H = sb([128, HC], i32)
        LU = sb([128, 1024], f32)
        OUTT = sb([128, 1024], f32)
        GA = sb([128, W1], i32)
        GB = sb([128, 10], i32)
        GC = sb([128, W3], i32)
        OFFS3 = sb([128, 1], i32)
        OFF = sb([128, 5], i32)
        VAL = sb([128, 5], f32)

        MS8F = sb([128, 8], f32)
        EQ1 = sb([128, 32], i32)
        LT1 = sb([128, 32], i32)
        M1 = sb([128, 32], i32)
        M1F = sb([128, 32], f32)
        SC1 = sb([128, 32], i32)
        SC3 = sb([128, 32], f32)
        SC4 = sb([128, 8], i32)
        SC5 = sb([128, 8], f32)
        EX = sb([128, 1], i32)
        BW2 = sb([128, 1], f32)
        BCONST = sb([128, 1], f32)
        TSID = sb([128, 1], i32)
        OFFT = sb([128, 1], i32)
        LTT = sb([128, 1], i32)
        LTTEX = sb([128, 1], i32)
        MT2 = sb([128, 1], i32)
        LT4 = sb([128, 4], i32)
        OFFBIB = sb([128, 4], i32)
        OFFBI2 = sb([128, 4], i32)
        EQALL = sb([128, 32], i32)
        COL = sb([128, 4], i32)
        COLE = sb([128, 4], i32)

        sem = lambda name: ctx.enter_context(nc.semaphore(name))
        sv = sem("sv")
        sg = sem("sg")
        sem_h = sem("sem_h")
        sem_lu = sem("sem_lu")
        sem_ga = sem("sem_ga")
        sem_gb = sem("sem_gb")
        sem_gc = sem("sem_gc")
        sem_out = sem("sem_out")
        sem_sc = sem("sem_sc")

        ctx.enter_context(nc.Block())

        g = nc.gpsimd
        v = nc.vector
        sy = nc.sync

        vcnt = [0]

        def vo(inst):
            if vcnt[0] > 0:
                inst.wait_op(sv, vcnt[0], "sem-ge")
            inst.then_inc(sv, 1)
            vcnt[0] += 1
            return inst

        def vw(*waits):
            for s_, val_ in waits:
                v.wait_ge(s_, val_)

        gcnt = [0]

        def go(inst):
            if gcnt[0] > 0:
                inst.wait_op(sg, gcnt[0], "sem-ge")
            inst.then_inc(sg, 1)
            gcnt[0] += 1
            return inst

        M_OFFS3 = 6
        M_OUTT = 11
        M_TSID = 18
        M_MT2 = 23
        M_ALL = 27

        # ================= sync: input DMAs + baseline write =================
        sy.dma_start(out=H[:, :], in_=hconst[:, :]).then_inc(sem_h, 16)
        sy.dma_start(out=LU[:, :], in_=lurep[:, :]).then_inc(sem_lu, 16)

        sy.wait_ge(sv, M_OUTT)
        sy.dma_start(
            out=outp[:, :].rearrange("(p f) o -> p (f o)", p=128),
            in_=OUTT[:, :],
        ).then_inc(sem_out, 16)

        # ================= gpsimd: 3 gathers + tail + 5 scatters =============
        def gather(dst, src, idx_ap, semh, *waits):
            for s_, val_ in waits:
                g.wait_ge(s_, val_)
            inst = g.indirect_dma_start(
                out=dst, out_offset=None,
                in_=src[:, :], in_offset=IndirectOffsetOnAxis(ap=idx_ap, axis=0),
            )
            inst.then_inc(semh, 16)
            return inst

        gather(GA[:, :], tu1, H[:, HC_IDXA : HC_IDXA + 1], sem_ga, (sem_h, 16))
        gather(GB[:, :], tu2, H[:, HC_IDXB : HC_IDXB + 1], sem_gb)
        gather(GC[:, :], tb, OFFS3[:, :], sem_gc, (sv, M_OFFS3))

        # trigram offset assembly (vector computes TSID + MT2)
        g.wait_ge(sv, M_TSID)
        go(g.tensor_add(OFFT[:, :], TSID[:, :], H[:, HC_OFFB2C : HC_OFFB2C + 1]))
        g.wait_ge(sv, M_MT2)
        go(g.tensor_add(OFF[:, 0:1], OFFT[:, :], MT2[:, :]))

        # correction scatters (after the baseline rows land in DRAM)
        g.wait_ge(sg, 2)
        g.wait_ge(sv, M_ALL)
        g.wait_ge(sem_out, 16)
        for col in range(5):
            g.indirect_dma_start(
                out=outp[:, :],
                out_offset=IndirectOffsetOnAxis(ap=OFF[:, col : col + 1], axis=0),
                in_=VAL[:, col : col + 1], in_offset=None,
                bounds_check=BOUNDS, oob_is_err=False,
            ).then_inc(sem_sc, 16)
        g.wait_ge(sem_sc, 80)

        # ================= vector =================
        # op 1
        vw((sem_h, 16))
        vo(v.tensor_copy(MS8F[:, :], H[:, HC_MS8 : HC_MS8 + 8]))

        # ops 2..6: find bigram node j -> TB offset (gates G_C)
        vw((sem_ga, 16))
        vo(
            v.tensor_tensor(
                EQ1[:, :], GA[:, A_C1 : A_C1 + 32],
                H[:, HC_H2 : HC_H2 + 1].to_broadcast([128, 32]), OP.is_equal,
            )
        )
        vo(
            v.tensor_tensor(
                LT1[:, :], H[:, HC_IOTA32 : HC_IOTA32 + 32],
                GA[:, A_NUM : A_NUM + 1].to_broadcast([128, 32]), OP.is_lt,
            )
        )
        vo(v.tensor_tensor(M1[:, :], EQ1[:, :], LT1[:, :], OP.logical_and))
        vo(v.tensor_tensor(SC1[:, :], M1[:, :], GA[:, A_TBC : A_TBC + 32], OP.mult))
        vo(v.tensor_reduce(OFFS3[:, :], SC1[:, :], axis=AX.X, op=OP.max))
        assert vcnt[0] == M_OFFS3

        # ops 7..11: bw2 select, bconst, dense baseline rows
        vo(v.tensor_copy(M1F[:, :], M1[:, :]))
        vo(
            v.tensor_tensor(
                SC3[:, :], M1F[:, :], GA[:, A_BW2 : A_BW2 + 32].bitcast(f32),
                OP.mult,
            )
        )
        vo(v.tensor_reduce(BW2[:, :], SC3[:, :], axis=AX.X, op=OP.min))
        vw((sem_gb, 16))
        vo(
            v.tensor_add(
                BCONST[:, :], GB[:, B_BW1 : B_BW1 + 1].bitcast(f32), BW2[:, :]
            )
        )
        vw((sem_lu, 16))
        vo(v.tensor_scalar(OUTT[:, :], LU[:, :], BCONST[:, 0:1], None, OP.add))
        assert vcnt[0] == M_OUTT

        # ops 12..16: bigram correction pre-work (G_C-wait window)
        vo(v.tensor_reduce(EX[:, :], M1[:, :], axis=AX.X, op=OP.max))
        vo(
            v.tensor_scalar(
                VAL[:, 1:5], GB[:, B_BL : B_BL + 4].bitcast(f32),
                BW2[:, 0:1], None, OP.add,
            )
        )
        vo(
            v.tensor_tensor(
                OFFBIB[:, :], GB[:, B_BI : B_BI + 4],
                H[:, HC_OFFBC : HC_OFFBC + 1].to_broadcast([128, 4]), OP.add,
            )
        )
        vo(
            v.tensor_tensor(
                LT4[:, :], H[:, HC_CI4 : HC_CI4 + 4],
                GB[:, B_NUM2 : B_NUM2 + 1].to_broadcast([128, 4]), OP.is_lt,
            )
        )
        vo(
            v.scalar_tensor_tensor(
                OFFBI2[:, :], LT4[:, :], -BIG, OFFBIB[:, :],
                op0=OP.mult, op1=OP.add,
            )
        )
        assert vcnt[0] == 16

        # ops 17..23: after G_C -- trigram slot selects + masks
        vw((sem_gc, 16))
        vo(v.tensor_tensor(SC4[:, :], H[:, HC_MS8 : HC_MS8 + 8], GC[:, C_TI : C_TI + 8], OP.mult))
        vo(v.tensor_reduce(TSID[:, :], SC4[:, :], axis=AX.X, op=OP.max))
        assert vcnt[0] == M_TSID
        vo(v.tensor_tensor(SC5[:, :], MS8F[:, :], GC[:, C_TL : C_TL + 8].bitcast(f32), OP.mult))
        vo(v.tensor_reduce(VAL[:, 0:1], SC5[:, :], axis=AX.X, op=OP.min))
        vo(
            v.tensor_tensor(
                LTT[:, :], H[:, HC_SCOL : HC_SCOL + 1],
                GC[:, C_NUM3 : C_NUM3 + 1], OP.is_lt,
            )
        )
        vo(v.tensor_add(LTTEX[:, :], LTT[:, :], EX[:, :]))
        vo(v.tensor_scalar(MT2[:, :], LTTEX[:, :], -BIG, None, OP.mult))
        assert vcnt[0] == M_MT2

        # ops 24..27: collision mask + final bigram offsets
        vo(
            v.tensor_tensor(
                EQALL[:, :].rearrange("p (q k) -> p q k", k=8),
                GB[:, B_BI : B_BI + 4].unsqueeze(2).to_broadcast([128, 4, 8]),
                GC[:, C_TI : C_TI + 8].unsqueeze(1).to_broadcast([128, 4, 8]),
                OP.is_equal,
            )
        )
        vo(
            v.tensor_reduce(
                COL[:, :], EQALL[:, :].rearrange("p (q k) -> p q k", k=8),
                axis=AX.X, op=OP.max,
            )
        )
        vo(
            v.tensor_tensor(
                COLE[:, :], COL[:, :], EX[:, 0:1].to_broadcast([128, 4]), OP.mult
            )
        )
        vo(
            v.scalar_tensor_tensor(
                OFF[:, 1:5], COLE[:, :], BIG, OFFBI2[:, :],
                op0=OP.mult, op1=OP.add,
            )
        )
        assert vcnt[0] == M_ALL

    return nc


def _build_tables(pointers, ids, logs):
    """Repack the (query-independent) trie into gather-friendly records."""
    ptr = np.asarray(pointers, dtype=np.int64)
    idsv = np.asarray(ids, dtype=np.int32)
    logsv = np.ascontiguousarray(np.asarray(logs, dtype=np.float32))
    logbits = logsv.view(np.int32)

    u = np.arange(U, dtype=np.int64)
    fc = u + ptr[:U]                         # first child (bigram node index)
    num = (ptr[1 : U + 1] - ptr[:U] + 1).astype(np.int64)

    cn = np.clip(fc[:, None] + np.arange(32)[None, :], U, U + B2 - 1)
    tu1 = np.zeros((U, W1), dtype=np.int32)
    tu1[:, A_NUM] = num.astype(np.int32)
    tu1[:, A_C1 : A_C1 + 32] = idsv[cn - U]
    tu1[:, A_TBC : A_TBC + 32] = ((cn - U) * W3).astype(np.int32)
    tu1[:, A_BW2 : A_BW2 + 32] = logbits[NNODES + cn]

    tu2 = np.zeros((U, W2), dtype=np.int32)
    bi_ids = idsv[cn - U]
    bi_logs = logbits[cn]
    for s in range(8):
        tu2[:, 10 * s + B_BI : 10 * s + B_BI + 4] = bi_ids[:, 4 * s : 4 * s + 4]
        tu2[:, 10 * s + B_BL : 10 * s + B_BL + 4] = bi_logs[:, 4 * s : 4 * s + 4]
        tu2[:, 10 * s + B_NUM2] = num.astype(np.int32)
        tu2[:, 10 * s + B_BW1] = logbits[NNODES + u]

    j = U + np.arange(B2, dtype=np.int64)
    fc3 = j + ptr[j]
    num3 = (ptr[j + 1] - ptr[j] + 1).astype(np.int32)
    cn3 = np.clip(fc3[:, None] + np.arange(8)[None, :], U + B2, NNODES - 1)
    tbl = np.zeros((B2, W3), dtype=np.int32)
    tbl[:, C_TI : C_TI + 8] = idsv[cn3 - U]
    tbl[:, C_TL : C_TL + 8] = logbits[cn3]
    tbl[:, C_NUM3] = num3

    lurep = np.ascontiguousarray(
        np.tile(logsv[:V].reshape(8, 1024), (16, 1)).astype(np.float32)
    )
    return (
        np.ascontiguousarray(tu1.reshape(U * W1, 1)),
        np.ascontiguousarray(tu2.reshape(U * W2, 1)),
        np.ascontiguousarray(tbl.reshape(B2 * W3, 1)),
        lurep,
    )


def _prep_in_maps(hist, idx, pointers, ids, logs):
    hist = np.asarray(hist)
    idxi = int(np.asarray(idx))
    hh = hist[:idxi][-(N - 1):]
    assert hh.shape == (2, BATCH), hh.shape
    tu1, tu2, tbl, lurep = _build_tables(pointers, ids, logs)

    p = np.arange(128)
    b = p >> 3
    s = p & 7
    hc_base = np.zeros((128, HC), dtype=np.int64)
    hc_base[:, HC_IOTA32 : HC_IOTA32 + 32] = np.arange(32)[None, :]
    hc_base[:, HC_MS8 : HC_MS8 + 8] = (np.arange(8)[None, :] == s[:, None])
    hc_base[:, HC_CI4 : HC_CI4 + 4] = (4 * s)[:, None] + np.arange(4)[None, :]
    hc_base[:, HC_OFFBC] = (b << 13) + BIG
    hc_base[:, HC_OFFB2C] = (b << 13) + 2 * BIG
    hc_base[:, HC_SCOL] = s

    in_maps = []
    for c in range(NCORES):
        sl = hh[:, c * BPC : (c + 1) * BPC].astype(np.int64)  # [2, 16]
        hc = hc_base.copy()
        h1 = sl[0][b]
        h2 = sl[1][b]
        hc[:, HC_IDXA] = h1 * W1
        hc[:, HC_IDXB] = h2 * W2 + 10 * s
        hc[:, HC_H2] = h2
        in_maps.append(
            {
                "hconst": np.ascontiguousarray(hc.astype(np.int32)),
                "lurep": lurep,
                "tu1": tu1,
                "tu2": tu2,
                "tb": tbl,
            }
        )
    return in_maps


def _assemble(results):
    return np.concatenate(
        [results[c]["out"].reshape(BPC, V) for c in range(NCORES)], axis=0
    )


def kernel(hist, idx, pointers, ids, logs):
    nc = build_kernel()
    in_maps = _prep_in_maps(hist, idx, pointers, ids, logs)
    res = run_bass_kernel_spmd(nc, in_maps, list(range(NCORES)))
    return _assemble(res.results)


def kernel_timed(hist, idx, pointers, ids, logs, trace=True):
    """Like kernel() but returns (output, BassKernelResults) with trace."""
    nc = build_kernel()
    in_maps = _prep_in_maps(hist, idx, pointers, ids, logs)
    res = run_bass_kernel_spmd(nc, in_maps, list(range(NCORES)), trace=trace)
    return _assemble(res.results), res
